# revision 1
# baseline (speedup 1.0000x reference)
"""Two-layer GCN encoder on 8 Trainium2 NeuronCores (Bass/Tile), v4.

  out = Anorm @ relu(Anorm @ (x@W1) + b1) @ W2 + b2,  Anorm = D^-1/2 (A+I) D^-1/2

Aggregation: per (src-quarter pass, dst-block) bucket, edge source rows are
fetched from the DRAM node table with ONE dma_gather (SWDGE, 4 rotating
queues, ~3ns/row desc-gen; trailing -1 idx pads generate no descriptors),
then scattered into PSUM with 0/1 one-hot matmuls (one batched is_equal per
bucket builds all slots).  Per-pass partial sums accumulate into an SBUF
accumulator; dinv[dst] and the bias are applied in the epilogue
(dinv per-partition, bias via a pre-broadcast [128, C] constant).
dinv[src] is folded into the table rows (x pre-scaled on the host) and
self-loops are ordinary edges.

Node tables are AllGathered in 4 quarter-collectives (f16 rows, 256B) that
overlap phase A / phase C and earlier aggregation passes.  The layer-2 table
is [*, 128] f16 with the top 64 columns unused, so its gather rows stay
256B.  Layer-1 output is PE-transposed per block for the phase-C matmul.
Single NEFF launch.
"""

import os

import numpy as np

import concourse.bass as bass
import concourse.bacc as bacc
import concourse.mybir as mybir
import concourse.tile as tile

P = 128
NQ = 4            # table quarters == aggregation passes

N_NODES = 100000
N_EDGES = 1600000
C_IN = 128
C_HID = 128
C_OUT = 64
N_CORES = 8


class Cfg:
    def __init__(self, n, cin, chid, cout, n_cores):
        assert n % n_cores == 0
        self.N = n
        self.CIN = cin
        self.CHID = chid
        self.COUT = cout
        self.NC = n_cores
        self.NPC = n // n_cores
        self.NBLK = -(-self.NPC // P)
        self.NPAD = self.NBLK * P
        assert self.NPC % NQ == 0
        self.QROWS = self.NPC // NQ          # local rows per quarter (3125)
        self.TROWS = self.QROWS * n_cores    # table rows per quarter (25000)
        assert self.TROWS <= 32767
        self.meta = None
        self.SCMAX = None
        self.ICSUM = None
        self.SSUM = None


def _wrap16(idx):
    """dma_gather idx layout: idx i -> [16k + i%16, i//16], replicated k=0..7."""
    n = idx.shape[0]
    assert n % 16 == 0
    ic = n // 16
    out = np.zeros((P, ic), np.int16)
    i = np.arange(n)
    for k in range(8):
        out[16 * k + (i % 16), i // 16] = idx
    return out


def prep_inputs(cfg, x, edge_index, W1, b1, W2, b2):
    """One SPMD program: per-(pass, block) slot counts are the MAX over cores;
    cores pad their buckets with dl=255 / idx=-1 (no DMA descriptors)."""
    NPC, QROWS = cfg.NPC, cfg.QROWS
    src = np.asarray(edge_index[0], dtype=np.int64)
    dst = np.asarray(edge_index[1], dtype=np.int64)
    deg = (np.bincount(dst, minlength=cfg.N) + 1.0).astype(np.float32)
    dinv = 1.0 / np.sqrt(deg)

    loops = np.arange(cfg.N, dtype=np.int64)
    src_all = np.concatenate([src, loops])
    dst_all = np.concatenate([dst, loops])
    order = np.argsort(dst_all, kind="stable")
    src_s = src_all[order]
    dst_s = dst_all[order]
    core_lo = np.searchsorted(dst_s, np.arange(cfg.NC) * NPC)
    core_hi = np.searchsorted(dst_s, (np.arange(cfg.NC) + 1) * NPC)

    x = np.asarray(x, dtype=np.float32)
    xs = x * dinv[:, None]
    W1 = np.asarray(W1, np.float32)
    b1 = np.asarray(b1, np.float32)
    W2 = np.asarray(W2, np.float32)
    b2 = np.asarray(b2, np.float32)

    nkey = NQ * cfg.NBLK
    per_core = []
    counts = np.zeros((cfg.NC, nkey), np.int64)
    for c in range(cfg.NC):
        lo, hi = core_lo[c], core_hi[c]
        s1 = src_s[lo:hi]
        d1 = dst_s[lo:hi] - c * NPC
        blk = (d1 >> 7).astype(np.int64)
        sl = s1 % NPC
        q = sl // QROWS
        tidx = (s1 // NPC) * QROWS + (sl % QROWS)
        key = q * cfg.NBLK + blk
        eorder = np.argsort(key, kind="stable")
        starts = np.searchsorted(key[eorder], np.arange(nkey + 1))
        counts[c] = starts[1:] - starts[:-1]
        per_core.append((tidx[eorder], (d1 & 127)[eorder], starts))

    sc = -(-counts.max(axis=0) // P)          # slots per (q, blk)
    sc = np.maximum(sc, 1)

    meta = []
    icoff = soff = 0
    for qq in range(NQ):
        for b in range(cfg.NBLK):
            s = int(sc[qq * cfg.NBLK + b])
            meta.append(dict(q=qq, b=b, sc=s, icoff=icoff, soff=soff))
            icoff += s * 8
            soff += s
    cfg.meta = meta
    cfg.SCMAX = int(sc.max())
    cfg.ICSUM = icoff
    cfg.SSUM = soff

    maps = []
    for c in range(cfg.NC):
        tidx_s, dloc_s, starts = per_core[c]
        idx_all = np.zeros((P, cfg.ICSUM), np.int16)
        dl_all = np.full((P, cfg.SSUM), 255.0, np.float16)
        for m in meta:
            k = m["q"] * cfg.NBLK + m["b"]
            a, bnd = starts[k], starts[k + 1]
            n = bnd - a
            W = m["sc"] * P
            ivv = np.zeros(W, np.int64)
            dvv = np.full(W, 255.0, np.float32)
            ivv[:n] = tidx_s[a:bnd]
            dvv[:n] = dloc_s[a:bnd]
            idx_all[:, m["icoff"]:m["icoff"] + W // 16] = \
                _wrap16(ivv.astype(np.int16))
            dl_all[:, m["soff"]:m["soff"] + m["sc"]] = \
                dvv.reshape(m["sc"], P).T.astype(np.float16)

        xsT = np.zeros((cfg.CIN, cfg.NPAD), np.float16)
        xsT[:, :NPC] = xs[c * NPC:(c + 1) * NPC].T
        dpad = np.ones(cfg.NPAD, np.float32)
        dpad[:NPC] = dinv[c * NPC:(c + 1) * NPC]
        dinvT = dpad.reshape(cfg.NBLK, P).T.copy()

        maps.append({
            "xsT": xsT,
            "dinvT": dinvT,
            "idx_all": idx_all,
            "dl_all": dl_all,
            "w1": W1.astype(np.float16),
            "w2": W2.astype(np.float16),
            "b1bc": np.tile(b1[None, :], (P, 1)).astype(np.float32),
            "b2bc": np.tile(b2[None, :], (P, 1)).astype(np.float32),
        })
    return maps


def _agg_layer(nc, cfg, pools, tab_q, idx_sb, dl_sb, iota_h, acc, ch, layer,
               qoff):
    """Per (pass, block): dma_gather bucket rows + 0/1 one-hot matmuls."""
    f32 = mybir.dt.float32
    f16 = mybir.dt.float16
    OP = mybir.AluOpType
    gtp, ohp, psp = pools
    meta = cfg.meta

    for q in range(NQ):
        for b in range(cfg.NBLK):
            m = meta[q * cfg.NBLK + b]
            sc = m["sc"]
            gt = gtp.tile([P, cfg.SCMAX, P], f16, tag=f"gt{layer}")
            nc.gpsimd.dma_gather(
                gt[:, :sc, :], tab_q[q][0:cfg.TROWS, :],
                idx_sb[:, m["icoff"]:m["icoff"] + sc * 8],
                sc * P, sc * P, P, elem_step=P,
                queue_num=(qoff + q * cfg.NBLK + b) % 4,
            )
            oh = ohp.tile([P, cfg.SCMAX, P], f16, tag=f"oh{layer}")
            nc.vector.tensor_tensor(
                out=oh[:, :sc, :],
                in0=iota_h[:].unsqueeze(1).to_broadcast([P, sc, P]),
                in1=dl_sb[:, m["soff"]:m["soff"] + sc]
                    .unsqueeze(2).to_broadcast([P, sc, P]),
                op=OP.is_equal,
            )
            ps = psp.tile([P, ch], f32, tag=f"ps{layer}", space="PSUM")
            for s in range(sc):
                nc.tensor.matmul(
                    out=ps[:], lhsT=oh[:, s, :], rhs=gt[:, s, :ch],
                    start=(s == 0), stop=(s == sc - 1),
                )
            nc.vector.tensor_tensor(
                out=acc[:, b, :], in0=ps[:], in1=acc[:, b, :], op=OP.add,
            )


def build_nc(cfg):
    nc = bacc.Bacc("TRN2", target_bir_lowering=False, debug=False,
                   num_devices=cfg.NC, num_swdge_queues=4)
    f32 = mybir.dt.float32
    f16 = mybir.dt.float16
    OP = mybir.AluOpType
    AF = mybir.ActivationFunctionType

    xsT = nc.dram_tensor("xsT", [cfg.CIN, cfg.NPAD], f16, kind="ExternalInput")
    dinvT = nc.dram_tensor("dinvT", [P, cfg.NBLK], f32, kind="ExternalInput")
    idx_t = nc.dram_tensor("idx_all", [P, cfg.ICSUM], mybir.dt.int16,
                           kind="ExternalInput")
    dl_t = nc.dram_tensor("dl_all", [P, cfg.SSUM], f16, kind="ExternalInput")
    w1 = nc.dram_tensor("w1", [cfg.CIN, cfg.CHID], f16, kind="ExternalInput")
    w2 = nc.dram_tensor("w2", [cfg.CHID, cfg.COUT], f16, kind="ExternalInput")
    b1bc = nc.dram_tensor("b1bc", [P, cfg.CHID], f32, kind="ExternalInput")
    b2bc = nc.dram_tensor("b2bc", [P, cfg.COUT], f32, kind="ExternalInput")
    out = nc.dram_tensor("out", [cfg.NPC, cfg.COUT], f32, kind="ExternalOutput")

    with tile.TileContext(nc) as tc:
        with (
            tc.tile_pool(name="const", bufs=1) as constp,
            tc.tile_pool(name="xt", bufs=3) as xtp,
            tc.tile_pool(name="st", bufs=4) as stp,
            tc.tile_pool(name="dram", bufs=1, space="DRAM") as dramp,
        ):
            iota_r = constp.tile([P, P], mybir.dt.int32)
            nc.gpsimd.iota(iota_r[:], pattern=[[1, P]], base=0,
                           channel_multiplier=0)
            iota_h = constp.tile([P, P], f16)
            nc.vector.tensor_copy(iota_h[:], iota_r[:])
            pidx = constp.tile([P, 1], mybir.dt.int32)
            nc.gpsimd.iota(pidx[:], pattern=[[1, 1]], base=0,
                           channel_multiplier=1)
            pidx_f = constp.tile([P, 1], f32)
            nc.vector.tensor_copy(pidx_f[:], pidx[:])
            identh = constp.tile([P, P], f16)
            nc.vector.tensor_scalar(out=identh[:], in0=iota_h[:],
                                    scalar1=pidx_f[:], scalar2=None,
                                    op0=OP.is_equal)

            w1b = constp.tile([cfg.CIN, cfg.CHID], f16)
            nc.sync.dma_start(w1b[:], w1.ap())
            w2b = constp.tile([cfg.CHID, cfg.COUT], f16)
            nc.sync.dma_start(w2b[:], w2.ap())
            b1c = constp.tile([P, cfg.CHID], f32)
            nc.sync.dma_start(b1c[:], b1bc.ap())
            b2c = constp.tile([P, cfg.COUT], f32)
            nc.sync.dma_start(b2c[:], b2bc.ap())
            dinv_sb = constp.tile([P, cfg.NBLK], f32)
            nc.sync.dma_start(dinv_sb[:], dinvT.ap())
            dl_sb = constp.tile([P, cfg.SSUM], f16)
            nc.sync.dma_start(dl_sb[:], dl_t.ap())
            idx_sb = constp.tile([P, cfg.ICSUM], mybir.dt.int16)
            nc.sync.dma_start(idx_sb[:], idx_t.ap())

            g1_own = [dramp.tile([cfg.QROWS, cfg.CHID], f16,
                                 name=f"g1_own{q}") for q in range(NQ)]
            tab1 = [dramp.tile([cfg.TROWS, cfg.CHID], f16,
                               addr_space="Shared", name=f"tab1_{q}")
                    for q in range(NQ)]
            g2_own = [dramp.tile([cfg.QROWS, P], f16,
                                 name=f"g2_own{q}") for q in range(NQ)]
            tab2 = [dramp.tile([cfg.TROWS, P], f16,
                               addr_space="Shared", name=f"tab2_{q}")
                    for q in range(NQ)]

            # ---- phase A: g1 = (dinv (.) x) @ W1, f16 rows -----------------
            qfired = 0
            with tc.tile_pool(name="psA", bufs=4, space="PSUM") as psp:
                for b in range(cfg.NBLK):
                    rows = min(P, cfg.NPC - b * P)
                    xt = xtp.tile([cfg.CIN, P], f16, tag="xt")
                    nc.sync.dma_start(xt[:], xsT.ap()[:, b * P:(b + 1) * P])
                    ps = psp.tile([P, cfg.CHID], f32, tag="psA", space="PSUM")
                    nc.tensor.matmul(out=ps[:], lhsT=xt[:], rhs=w1b[:],
                                     start=True, stop=True)
                    st = stp.tile([P, cfg.CHID], f16, tag="stA")
                    nc.scalar.activation(st[:], ps[:], AF.Copy)
                    lo = b * P
                    hi = b * P + rows
                    while lo < hi:
                        q = lo // cfg.QROWS
                        qe = min(hi, (q + 1) * cfg.QROWS)
                        nc.sync.dma_start(
                            g1_own[q][lo - q * cfg.QROWS: qe - q * cfg.QROWS,
                                      :],
                            st[lo - b * P: qe - b * P, :])
                        lo = qe
                    while (qfired < NQ
                           and b * P + rows >= (qfired + 1) * cfg.QROWS):
                        nc.gpsimd.collective_compute(
                            "AllGather", OP.bypass,
                            replica_groups=[list(range(cfg.NC))],
                            ins=[g1_own[qfired].opt()],
                            outs=[tab1[qfired].opt()],
                        )
                        qfired += 1

            # ---- layer-1 aggregation --------------------------------------
            with (
                tc.tile_pool(name="hacc", bufs=1) as haccp,
                tc.tile_pool(name="gt1", bufs=6) as gtp,
                tc.tile_pool(name="oh1", bufs=4) as ohp,
                tc.tile_pool(name="ps1", bufs=4, space="PSUM") as psp1,
            ):
                h_acc = haccp.tile([P, cfg.NBLK, cfg.CHID], f32)
                nc.vector.memset(h_acc[:], 0.0)
                for _w in range(6):
                    gw = gtp.tile([P, cfg.SCMAX, P], f16, tag="gt1")
                    nc.vector.memset(gw[:], 0.0)
                _agg_layer(nc, cfg, (gtp, ohp, psp1), tab1, idx_sb[:],
                           dl_sb[:], iota_h, h_acc, cfg.CHID, 1, 0)

                # epilogue + hT + phase C + AG2, quarter-chunked
                with (
                    tc.tile_pool(name="epi", bufs=4) as epip,
                    tc.tile_pool(name="psT", bufs=2, space="PSUM") as pspT,
                    tc.tile_pool(name="psC", bufs=2, space="PSUM") as pspC,
                ):
                    ch_blocks = [(0, 25), (25, 50), (50, 75), (75, cfg.NBLK)]
                    for q, (blo, bhi) in enumerate(ch_blocks):
                        for b in range(blo, bhi):
                            rows = min(P, cfg.NPC - b * P)
                            t1 = epip.tile([P, cfg.CHID], f32, tag="t1")
                            nc.vector.scalar_tensor_tensor(
                                out=t1[:], in0=h_acc[:, b, :],
                                scalar=dinv_sb[:, b:b + 1], in1=b1c[:],
                                op0=OP.mult, op1=OP.add)
                            hb = epip.tile([P, cfg.CHID], f16, tag="hb")
                            nc.vector.tensor_scalar(
                                out=hb[:], in0=t1[:],
                                scalar1=0.0, scalar2=dinv_sb[:, b:b + 1],
                                op0=OP.max, op1=OP.mult)
                            pst = pspT.tile([P, cfg.CHID], f16, tag="pst",
                                            space="PSUM")
                            nc.tensor.matmul(out=pst[:], lhsT=hb[:],
                                             rhs=identh[:], start=True,
                                             stop=True, is_transpose=True)
                            hTb = epip.tile([cfg.CHID, P], f16, tag="hTb")
                            nc.scalar.activation(hTb[:], pst[:], AF.Copy)
                            ps = pspC.tile([P, cfg.COUT], f32, tag="psC",
                                           space="PSUM")
                            nc.tensor.matmul(out=ps[:], lhsT=hTb[:],
                                             rhs=w2b[:], start=True, stop=True)
                            st = stp.tile([P, P], f16, tag="stC")
                            nc.scalar.activation(st[:, :cfg.COUT], ps[:],
                                                 AF.Copy)
                            lo = b * P
                            hi = b * P + rows
                            while lo < hi:
                                qq = lo // cfg.QROWS
                                qe = min(hi, (qq + 1) * cfg.QROWS)
                                nc.sync.dma_start(
                                    g2_own[qq][lo - qq * cfg.QROWS:
                                               qe - qq * cfg.QROWS, :cfg.COUT],
                                    st[lo - b * P: qe - b * P, :cfg.COUT])
                                lo = qe
                        nc.gpsimd.collective_compute(
                            "AllGather", OP.bypass,
                            replica_groups=[list(range(cfg.NC))],
                            ins=[g2_own[q].opt()], outs=[tab2[q].opt()],
                        )

            # ---- layer-2 aggregation --------------------------------------
            with (
                tc.tile_pool(name="acc2p", bufs=1) as acc2p,
                tc.tile_pool(name="gt2", bufs=6) as gtp2,
                tc.tile_pool(name="oh2", bufs=4) as ohp2,
                tc.tile_pool(name="ps2", bufs=8, space="PSUM") as psp2,
                tc.tile_pool(name="oute", bufs=4) as outep,
            ):
                acc2 = acc2p.tile([P, cfg.NBLK, cfg.COUT], f32)
                nc.vector.memset(acc2[:], 0.0)
                for _w in range(6):
                    gw = gtp2.tile([P, cfg.SCMAX, P], f16, tag="gt2")
                    nc.vector.memset(gw[:], 0.0)
                _agg_layer(nc, cfg, (gtp2, ohp2, psp2), tab2, idx_sb[:],
                           dl_sb[:], iota_h, acc2, cfg.COUT, 2, 2)
                for b in range(cfg.NBLK):
                    rows = min(P, cfg.NPC - b * P)
                    ot = outep.tile([P, cfg.COUT], f32, tag="ot")
                    nc.vector.scalar_tensor_tensor(
                        out=ot[:], in0=acc2[:, b, :],
                        scalar=dinv_sb[:, b:b + 1], in1=b2c[:],
                        op0=OP.mult, op1=OP.add)
                    nc.sync.dma_start(out.ap()[b * P: b * P + rows, :],
                                      ot[:rows, :])

    nc.compile()
    return nc


def run_cfg(cfg, inputs, ncs=None):
    from concourse import bass_utils

    maps = prep_inputs(
        cfg, inputs["x"], inputs["edge_index"], inputs["W1"], inputs["b1"],
        inputs["W2"], inputs["b2"],
    )
    nc = ncs if ncs else build_nc(cfg)

    kwargs = {}
    if os.environ.get("GCN_TRACE"):
        base = os.environ.get("GCN_TMPDIR")
        if base:
            os.makedirs(base, exist_ok=True)
        kwargs = dict(trace=True, tmpdir=base)

    res = bass_utils.run_bass_kernel_spmd(
        nc, maps, core_ids=list(range(cfg.NC)), **kwargs
    )
    outp = np.concatenate([res.results[c]["out"] for c in range(cfg.NC)],
                          axis=0)
    t = res.exec_time_ns
    return outp.astype(np.float32), (t, t, None)


def kernel(**inputs):
    cfg = Cfg(N_NODES, C_IN, C_HID, C_OUT, N_CORES)
    outp, _ = run_cfg(cfg, inputs)
    return outp



# revision 6
# speedup vs baseline: 1.0301x; 1.0301x over previous
"""Two-layer GCN encoder on 8 Trainium2 NeuronCores (Bass/Tile), v4.

  out = Anorm @ relu(Anorm @ (x@W1) + b1) @ W2 + b2,  Anorm = D^-1/2 (A+I) D^-1/2

Aggregation: per (src-quarter pass, dst-block) bucket, edge source rows are
fetched from the DRAM node table with ONE dma_gather (SWDGE, 4 rotating
queues, ~3ns/row desc-gen; trailing -1 idx pads generate no descriptors),
then scattered into PSUM with 0/1 one-hot matmuls (one batched is_equal per
bucket builds all slots).  Per-pass partial sums accumulate into an SBUF
accumulator; dinv[dst] and the bias are applied in the epilogue
(dinv per-partition, bias via a pre-broadcast [128, C] constant).
dinv[src] is folded into the table rows (x pre-scaled on the host) and
self-loops are ordinary edges.

Node tables are AllGathered in 4 quarter-collectives (f16 rows, 256B) that
overlap phase A / phase C and earlier aggregation passes.  The layer-2 table
is [*, 128] f16 with the top 64 columns unused, so its gather rows stay
256B.  Layer-1 output is PE-transposed per block for the phase-C matmul.
Single NEFF launch.
"""

import os

import numpy as np

import concourse.bass as bass
import concourse.bacc as bacc
import concourse.mybir as mybir
import concourse.tile as tile

P = 128
NQ = 4            # table quarters == aggregation passes

N_NODES = 100000
N_EDGES = 1600000
C_IN = 128
C_HID = 128
C_OUT = 64
N_CORES = 8


class Cfg:
    def __init__(self, n, cin, chid, cout, n_cores):
        assert n % n_cores == 0
        self.N = n
        self.CIN = cin
        self.CHID = chid
        self.COUT = cout
        self.NC = n_cores
        self.NPC = n // n_cores
        self.NBLK = -(-self.NPC // P)
        self.NPAD = self.NBLK * P
        assert self.NPC % NQ == 0
        self.QROWS = self.NPC // NQ          # local rows per quarter (3125)
        self.TROWS = self.QROWS * n_cores    # table rows per quarter (25000)
        assert self.TROWS <= 32767
        self.meta = None
        self.SCMAX = None
        self.ICSUM = None
        self.SSUM = None


def _wrap16(idx):
    """dma_gather idx layout: idx i -> [16k + i%16, i//16], replicated k=0..7."""
    n = idx.shape[0]
    assert n % 16 == 0
    ic = n // 16
    out = np.zeros((P, ic), np.int16)
    i = np.arange(n)
    for k in range(8):
        out[16 * k + (i % 16), i // 16] = idx
    return out


def prep_inputs(cfg, x, edge_index, W1, b1, W2, b2):
    """One SPMD program: per-(pass, block) slot counts are the MAX over cores;
    cores pad their buckets with dl=255 / idx=-1 (no DMA descriptors)."""
    NPC, QROWS = cfg.NPC, cfg.QROWS
    src = np.asarray(edge_index[0], dtype=np.int64)
    dst = np.asarray(edge_index[1], dtype=np.int64)
    deg = (np.bincount(dst, minlength=cfg.N) + 1.0).astype(np.float32)
    dinv = 1.0 / np.sqrt(deg)

    # self-loops are handled locally in the epilogues (dinv * g rows),
    # not routed through the gather path
    src_all = src
    dst_all = dst
    order = np.argsort(dst_all, kind="stable")
    src_s = src_all[order]
    dst_s = dst_all[order]
    core_lo = np.searchsorted(dst_s, np.arange(cfg.NC) * NPC)
    core_hi = np.searchsorted(dst_s, (np.arange(cfg.NC) + 1) * NPC)

    x = np.asarray(x, dtype=np.float32)
    xs = x * dinv[:, None]
    W1 = np.asarray(W1, np.float32)
    b1 = np.asarray(b1, np.float32)
    W2 = np.asarray(W2, np.float32)
    b2 = np.asarray(b2, np.float32)

    nkey = NQ * cfg.NBLK
    per_core = []
    counts = np.zeros((cfg.NC, nkey), np.int64)
    for c in range(cfg.NC):
        lo, hi = core_lo[c], core_hi[c]
        s1 = src_s[lo:hi]
        d1 = dst_s[lo:hi] - c * NPC
        blk = (d1 >> 7).astype(np.int64)
        sl = s1 % NPC
        q = sl // QROWS
        tidx = (s1 // NPC) * QROWS + (sl % QROWS)
        key = q * cfg.NBLK + blk
        eorder = np.argsort(key, kind="stable")
        starts = np.searchsorted(key[eorder], np.arange(nkey + 1))
        counts[c] = starts[1:] - starts[:-1]
        per_core.append((tidx[eorder], (d1 & 127)[eorder], starts))

    sc = -(-counts.max(axis=0) // P)          # slots per (q, blk)
    sc = np.maximum(sc, 1)

    meta = []
    icoff = soff = 0
    for qq in range(NQ):
        for b in range(cfg.NBLK):
            s = int(sc[qq * cfg.NBLK + b])
            meta.append(dict(q=qq, b=b, sc=s, icoff=icoff, soff=soff))
            icoff += s * 8
            soff += s
    cfg.meta = meta
    cfg.SCMAX = int(sc.max())
    cfg.ICSUM = icoff
    cfg.SSUM = soff

    maps = []
    for c in range(cfg.NC):
        tidx_s, dloc_s, starts = per_core[c]
        idx_all = np.zeros((P, cfg.ICSUM), np.int16)
        dl_all = np.full((P, cfg.SSUM), 255.0, np.float16)
        for m in meta:
            k = m["q"] * cfg.NBLK + m["b"]
            a, bnd = starts[k], starts[k + 1]
            n = bnd - a
            W = m["sc"] * P
            ivv = np.zeros(W, np.int64)
            dvv = np.full(W, 255.0, np.float32)
            ivv[:n] = tidx_s[a:bnd]
            dvv[:n] = dloc_s[a:bnd]
            idx_all[:, m["icoff"]:m["icoff"] + W // 16] = \
                _wrap16(ivv.astype(np.int16))
            dl_all[:, m["soff"]:m["soff"] + m["sc"]] = \
                dvv.reshape(m["sc"], P).T.astype(np.float16)

        xsT = np.zeros((cfg.CIN, cfg.NPAD), np.float16)
        xsT[:, :NPC] = xs[c * NPC:(c + 1) * NPC].T
        dpad = np.ones(cfg.NPAD, np.float32)
        dpad[:NPC] = dinv[c * NPC:(c + 1) * NPC]
        dinvT = dpad.reshape(cfg.NBLK, P).T.copy()

        maps.append({
            "xsT": xsT,
            "dinvT": dinvT,
            "idx_all": idx_all,
            "dl_all": dl_all,
            "w1": W1.astype(np.float16),
            "w2": W2.astype(np.float16),
            "b1bc": np.tile(b1[None, :], (P, 1)).astype(np.float32),
            "b2bc": np.tile(b2[None, :], (P, 1)).astype(np.float32),
        })
    return maps


def _agg_layer(nc, cfg, pools, tab_q, idx_sb, dl_sb, iota_h, acc, ch, layer,
               qoff):
    """Per (pass, block): dma_gather bucket rows + 0/1 one-hot matmuls."""
    f32 = mybir.dt.float32
    f16 = mybir.dt.float16
    OP = mybir.AluOpType
    gtp, ohp, psp = pools
    meta = cfg.meta

    for q in range(NQ):
        for b in range(cfg.NBLK):
            m = meta[q * cfg.NBLK + b]
            sc = m["sc"]
            gt = gtp.tile([P, cfg.SCMAX, P], f16, tag=f"gt{layer}")
            nc.gpsimd.dma_gather(
                gt[:, :sc, :], tab_q[q][0:cfg.TROWS, :],
                idx_sb[:, m["icoff"]:m["icoff"] + sc * 8],
                sc * P, sc * P, P, elem_step=P,
                queue_num=(qoff + q * cfg.NBLK + b) % 4,
            )
            oh = ohp.tile([P, cfg.SCMAX, P], f16, tag=f"oh{layer}")
            nc.vector.tensor_tensor(
                out=oh[:, :sc, :],
                in0=iota_h[:].unsqueeze(1).to_broadcast([P, sc, P]),
                in1=dl_sb[:, m["soff"]:m["soff"] + sc]
                    .unsqueeze(2).to_broadcast([P, sc, P]),
                op=OP.is_equal,
            )
            ps = psp.tile([P, ch], f32, tag=f"ps{layer}", space="PSUM")
            for s in range(sc):
                nc.tensor.matmul(
                    out=ps[:], lhsT=oh[:, s, :], rhs=gt[:, s, :ch],
                    start=(s == 0), stop=(s == sc - 1),
                )
            nc.vector.tensor_tensor(
                out=acc[:, b, :], in0=ps[:], in1=acc[:, b, :], op=OP.add,
            )


def build_nc(cfg):
    nc = bacc.Bacc("TRN2", target_bir_lowering=False, debug=False,
                   num_devices=cfg.NC, num_swdge_queues=4)
    f32 = mybir.dt.float32
    f16 = mybir.dt.float16
    OP = mybir.AluOpType
    AF = mybir.ActivationFunctionType

    xsT = nc.dram_tensor("xsT", [cfg.CIN, cfg.NPAD], f16, kind="ExternalInput")
    dinvT = nc.dram_tensor("dinvT", [P, cfg.NBLK], f32, kind="ExternalInput")
    idx_t = nc.dram_tensor("idx_all", [P, cfg.ICSUM], mybir.dt.int16,
                           kind="ExternalInput")
    dl_t = nc.dram_tensor("dl_all", [P, cfg.SSUM], f16, kind="ExternalInput")
    w1 = nc.dram_tensor("w1", [cfg.CIN, cfg.CHID], f16, kind="ExternalInput")
    w2 = nc.dram_tensor("w2", [cfg.CHID, cfg.COUT], f16, kind="ExternalInput")
    b1bc = nc.dram_tensor("b1bc", [P, cfg.CHID], f32, kind="ExternalInput")
    b2bc = nc.dram_tensor("b2bc", [P, cfg.COUT], f32, kind="ExternalInput")
    out = nc.dram_tensor("out", [cfg.NPC, cfg.COUT], f32, kind="ExternalOutput")

    with tile.TileContext(nc) as tc:
        with (
            tc.tile_pool(name="const", bufs=1) as constp,
            tc.tile_pool(name="xt", bufs=3) as xtp,
            tc.tile_pool(name="st", bufs=4) as stp,
            tc.tile_pool(name="dram", bufs=1, space="DRAM") as dramp,
        ):
            iota_r = constp.tile([P, P], mybir.dt.int32)
            nc.gpsimd.iota(iota_r[:], pattern=[[1, P]], base=0,
                           channel_multiplier=0)
            iota_h = constp.tile([P, P], f16)
            nc.vector.tensor_copy(iota_h[:], iota_r[:])
            pidx = constp.tile([P, 1], mybir.dt.int32)
            nc.gpsimd.iota(pidx[:], pattern=[[1, 1]], base=0,
                           channel_multiplier=1)
            pidx_f = constp.tile([P, 1], f32)
            nc.vector.tensor_copy(pidx_f[:], pidx[:])
            identh = constp.tile([P, P], f16)
            nc.vector.tensor_scalar(out=identh[:], in0=iota_h[:],
                                    scalar1=pidx_f[:], scalar2=None,
                                    op0=OP.is_equal)

            w1b = constp.tile([cfg.CIN, cfg.CHID], f16)
            nc.sync.dma_start(w1b[:], w1.ap())
            w2b = constp.tile([cfg.CHID, cfg.COUT], f16)
            nc.sync.dma_start(w2b[:], w2.ap())
            b1c = constp.tile([P, cfg.CHID], f32)
            nc.sync.dma_start(b1c[:], b1bc.ap())
            b2c = constp.tile([P, cfg.COUT], f32)
            nc.sync.dma_start(b2c[:], b2bc.ap())
            dinv_sb = constp.tile([P, cfg.NBLK], f32)
            nc.sync.dma_start(dinv_sb[:], dinvT.ap())
            dl_sb = constp.tile([P, cfg.SSUM], f16)
            nc.sync.dma_start(dl_sb[:], dl_t.ap())
            idx_sb = constp.tile([P, cfg.ICSUM], mybir.dt.int16)
            nc.sync.dma_start(idx_sb[:], idx_t.ap())

            g1_own = [dramp.tile([cfg.QROWS, cfg.CHID], f16,
                                 name=f"g1_own{q}") for q in range(NQ)]
            tab1 = [dramp.tile([cfg.TROWS, cfg.CHID], f16,
                               addr_space="Shared", name=f"tab1_{q}")
                    for q in range(NQ)]
            g2_own = [dramp.tile([cfg.QROWS, P], f16,
                                 name=f"g2_own{q}") for q in range(NQ)]
            tab2 = [dramp.tile([cfg.TROWS, P], f16,
                               addr_space="Shared", name=f"tab2_{q}")
                    for q in range(NQ)]

            # persistent local copies of g1/g2 rows for the self-loop terms
            g1k = constp.tile([P, cfg.NBLK, cfg.CHID], f16)
            g2k = constp.tile([P, cfg.NBLK, cfg.COUT], f16)

            # ---- phase A: g1 = (dinv (.) x) @ W1, f16 rows -----------------
            qfired = 0
            with tc.tile_pool(name="psA", bufs=4, space="PSUM") as psp:
                for b in range(cfg.NBLK):
                    rows = min(P, cfg.NPC - b * P)
                    xt = xtp.tile([cfg.CIN, P], f16, tag="xt")
                    nc.sync.dma_start(xt[:], xsT.ap()[:, b * P:(b + 1) * P])
                    ps = psp.tile([P, cfg.CHID], f32, tag="psA", space="PSUM")
                    nc.tensor.matmul(out=ps[:], lhsT=xt[:], rhs=w1b[:],
                                     start=True, stop=True)
                    nc.scalar.activation(g1k[:, b, :], ps[:], AF.Copy)
                    lo = b * P
                    hi = b * P + rows
                    while lo < hi:
                        q = lo // cfg.QROWS
                        qe = min(hi, (q + 1) * cfg.QROWS)
                        nc.sync.dma_start(
                            g1_own[q][lo - q * cfg.QROWS: qe - q * cfg.QROWS,
                                      :],
                            g1k[lo - b * P: qe - b * P, b, :])
                        lo = qe
                    while (qfired < NQ
                           and b * P + rows >= (qfired + 1) * cfg.QROWS):
                        nc.gpsimd.collective_compute(
                            "AllGather", OP.bypass,
                            replica_groups=[list(range(cfg.NC))],
                            ins=[g1_own[qfired].opt()],
                            outs=[tab1[qfired].opt()],
                        )
                        qfired += 1

            # ---- layer-1 aggregation --------------------------------------
            with (
                tc.tile_pool(name="hacc", bufs=1) as haccp,
                tc.tile_pool(name="gt1", bufs=6) as gtp,
                tc.tile_pool(name="oh1", bufs=4) as ohp,
                tc.tile_pool(name="ps1", bufs=4, space="PSUM") as psp1,
            ):
                h_acc = haccp.tile([P, cfg.NBLK, cfg.CHID], f32)
                nc.vector.memset(h_acc[:], 0.0)
                for _w in range(6):
                    gw = gtp.tile([P, cfg.SCMAX, P], f16, tag="gt1")
                    nc.vector.memset(gw[:], 0.0)
                _agg_layer(nc, cfg, (gtp, ohp, psp1), tab1, idx_sb[:],
                           dl_sb[:], iota_h, h_acc, cfg.CHID, 1, 0)

                # epilogue + hT + phase C + AG2, quarter-chunked
                with (
                    tc.tile_pool(name="epi", bufs=4) as epip,
                    tc.tile_pool(name="psT", bufs=2, space="PSUM") as pspT,
                    tc.tile_pool(name="psC", bufs=2, space="PSUM") as pspC,
                ):
                    ch_blocks = [(0, 25), (25, 50), (50, 75), (75, cfg.NBLK)]
                    for q, (blo, bhi) in enumerate(ch_blocks):
                        for b in range(blo, bhi):
                            rows = min(P, cfg.NPC - b * P)
                            nc.vector.tensor_tensor(
                                out=h_acc[:, b, :], in0=h_acc[:, b, :],
                                in1=g1k[:, b, :], op=OP.add)
                            t1 = epip.tile([P, cfg.CHID], f32, tag="t1")
                            nc.vector.scalar_tensor_tensor(
                                out=t1[:], in0=h_acc[:, b, :],
                                scalar=dinv_sb[:, b:b + 1], in1=b1c[:],
                                op0=OP.mult, op1=OP.add)
                            hb = epip.tile([P, cfg.CHID], f16, tag="hb")
                            nc.vector.tensor_scalar(
                                out=hb[:], in0=t1[:],
                                scalar1=0.0, scalar2=dinv_sb[:, b:b + 1],
                                op0=OP.max, op1=OP.mult)
                            pst = pspT.tile([P, cfg.CHID], f16, tag="pst",
                                            space="PSUM")
                            nc.tensor.matmul(out=pst[:], lhsT=hb[:],
                                             rhs=identh[:], start=True,
                                             stop=True, is_transpose=True)
                            hTb = epip.tile([cfg.CHID, P], f16, tag="hTb")
                            nc.scalar.activation(hTb[:], pst[:], AF.Copy)
                            ps = pspC.tile([P, cfg.COUT], f32, tag="psC",
                                           space="PSUM")
                            nc.tensor.matmul(out=ps[:], lhsT=hTb[:],
                                             rhs=w2b[:], start=True, stop=True)
                            nc.scalar.activation(g2k[:, b, :], ps[:],
                                                 AF.Copy)
                            lo = b * P
                            hi = b * P + rows
                            while lo < hi:
                                qq = lo // cfg.QROWS
                                qe = min(hi, (qq + 1) * cfg.QROWS)
                                nc.sync.dma_start(
                                    g2_own[qq][lo - qq * cfg.QROWS:
                                               qe - qq * cfg.QROWS, :cfg.COUT],
                                    g2k[lo - b * P: qe - b * P, b, :])
                                lo = qe
                        nc.gpsimd.collective_compute(
                            "AllGather", OP.bypass,
                            replica_groups=[list(range(cfg.NC))],
                            ins=[g2_own[q].opt()], outs=[tab2[q].opt()],
                        )

            # ---- layer-2 aggregation --------------------------------------
            with (
                tc.tile_pool(name="acc2p", bufs=1) as acc2p,
                tc.tile_pool(name="gt2", bufs=6) as gtp2,
                tc.tile_pool(name="oh2", bufs=4) as ohp2,
                tc.tile_pool(name="ps2", bufs=8, space="PSUM") as psp2,
                tc.tile_pool(name="oute", bufs=4) as outep,
            ):
                acc2 = acc2p.tile([P, cfg.NBLK, cfg.COUT], f32)
                nc.vector.memset(acc2[:], 0.0)
                for _w in range(6):
                    gw = gtp2.tile([P, cfg.SCMAX, P], f16, tag="gt2")
                    nc.vector.memset(gw[:], 0.0)
                _agg_layer(nc, cfg, (gtp2, ohp2, psp2), tab2, idx_sb[:],
                           dl_sb[:], iota_h, acc2, cfg.COUT, 2, 2)
                for b in range(cfg.NBLK):
                    rows = min(P, cfg.NPC - b * P)
                    nc.vector.tensor_tensor(
                        out=acc2[:, b, :], in0=acc2[:, b, :],
                        in1=g2k[:, b, :], op=OP.add)
                    ot = outep.tile([P, cfg.COUT], f32, tag="ot")
                    nc.vector.scalar_tensor_tensor(
                        out=ot[:], in0=acc2[:, b, :],
                        scalar=dinv_sb[:, b:b + 1], in1=b2c[:],
                        op0=OP.mult, op1=OP.add)
                    nc.sync.dma_start(out.ap()[b * P: b * P + rows, :],
                                      ot[:rows, :])

    nc.compile()
    return nc


def run_cfg(cfg, inputs, ncs=None):
    from concourse import bass_utils

    maps = prep_inputs(
        cfg, inputs["x"], inputs["edge_index"], inputs["W1"], inputs["b1"],
        inputs["W2"], inputs["b2"],
    )
    nc = ncs if ncs else build_nc(cfg)

    kwargs = {}
    if os.environ.get("GCN_TRACE"):
        base = os.environ.get("GCN_TMPDIR")
        if base:
            os.makedirs(base, exist_ok=True)
        kwargs = dict(trace=True, tmpdir=base)

    res = bass_utils.run_bass_kernel_spmd(
        nc, maps, core_ids=list(range(cfg.NC)), **kwargs
    )
    outp = np.concatenate([res.results[c]["out"] for c in range(cfg.NC)],
                          axis=0)
    t = res.exec_time_ns
    return outp.astype(np.float32), (t, t, None)


def kernel(**inputs):
    cfg = Cfg(N_NODES, C_IN, C_HID, C_OUT, N_CORES)
    outp, _ = run_cfg(cfg, inputs)
    return outp

